# revision 1
# baseline (speedup 1.0000x reference)
"""Multi-head attention (B=2, H=8, S=2048, hd=16) on 8 Trainium2 NeuronCores.

Sharding: 16 (batch, head) attention groups -> 2 heads per core (cores 0-3:
batch 0, cores 4-7: batch 1).  Each core receives the (transposed) embeddings
for its batch, the 32 projection-weight columns for its two heads, and a
key-compacted copy of the embeddings (keys whose source mask is 0 contribute
exactly-zero softmax probability in fp32, so they are dropped; the compacted
set is padded to NK=1280 with -1000 additive-mask columns which also exp to
exactly 0).

Per head the kernel runs a two-pass softmax:
  pass A ([q,k] layout): S = (Q/4)K^T + mask via a 17-row contraction
    (16 dims + ones*mask row); DVE reduce_max(negate) gives -rowmax.
  pass B ([k,q] layout): S^T - rowmax via an 18-row contraction (16 dims +
    mask*ones + ones*(-rowmax)); ACT exp -> P^T in SBUF.
  ctx: P^T @ [V | 1] accumulated in PSUM with col-packed (tile_position)
    matmuls; the ones column yields the softmax denominator l. Final scale by
    1/l uses a gpsimd partition-broadcast + DVE multiply.

Output per core is a dense [32, 2048] (dim-major) tensor; the host scatters
columns back into the interleaved head layout (out[..., d*8+h] = ctx[d]).
"""

import numpy as np

S = 2048
E = 128
HD = 16
NK = 1280            # padded compacted key count (binomial(2048,1/2) + 11 sigma)
NKB = NK // 128      # 10 key blocks
NQB = S // 128       # 16 query blocks
NEG = -1000.0

_PROG = None


def _build_program():
    import concourse.mybir as mybir
    from concourse import bacc
    from concourse.tile import TileContext

    fp32 = mybir.dt.float32
    # float32r matmuls measured 1.2e-2 rel error on HW - too imprecise for
    # the score/ctx path; keep those exact fp32. The MAX pass is immune to
    # operand rounding (the bias cancels in normalization), so it runs on
    # real-f32r copies of Q/K at 1 cycle/row instead of 4.
    f32r = mybir.dt.float32
    f32rr = mybir.dt.float32r
    AF = mybir.ActivationFunctionType
    ALU = mybir.AluOpType
    AX = mybir.AxisListType

    nc = bacc.Bacc()

    xT = nc.declare_dram_parameter("xT", [E, S], f32r, isOutput=False)
    xkT = nc.declare_dram_parameter("xkT", [E, NK], f32r, isOutput=False)
    # weight columns padded to 48: head0 dims at 0:16, head1 dims at 32:48
    # (PSUM partition slices must start at 0/32/64/96)
    wq = nc.declare_dram_parameter("wq", [E, 48], f32r, isOutput=False)
    wk = nc.declare_dram_parameter("wk", [E, 48], f32r, isOutput=False)
    wv = nc.declare_dram_parameter("wv", [E, 48], f32r, isOutput=False)
    maskrow = nc.declare_dram_parameter("maskrow", [1, NK], f32r, isOutput=False)
    onesrow = nc.declare_dram_parameter("onesrow", [1, S], f32r, isOutput=False)
    onesr = nc.declare_dram_parameter("onesr", [1, S], f32rr, isOutput=False)
    maskr = nc.declare_dram_parameter("maskr", [1, NK], f32rr, isOutput=False)
    ident = nc.declare_dram_parameter("ident", [E, E], fp32, isOutput=False)
    out_d = nc.declare_dram_parameter("out", [2 * HD, S], fp32, isOutput=True)
    ldram = nc.dram_tensor("ldram", [2, S], fp32)

    with TileContext(nc) as tc:
        with (
            tc.tile_pool(name="consts", bufs=1) as cpool,
            tc.tile_pool(name="work", bufs=1) as wpool,
            tc.tile_pool(name="ptp", bufs=3) as ptpool,
            tc.tile_pool(name="stp", bufs=2, space="PSUM") as stpool,
            tc.tile_pool(name="ap", bufs=2, space="PSUM") as apool,
            tc.tile_pool(name="ctxp", bufs=2, space="PSUM") as ctxpool,
        ):
            # ---------------- constant loads ----------------
            xT_sb = cpool.tile([E, S], f32r, name="xT_sb")
            nc.sync.dma_start(out=xT_sb[:, :], in_=xT[:, :])
            xkT_sb = cpool.tile([E, NK], f32r, name="xkT_sb")
            nc.sync.dma_start(out=xkT_sb[:, :], in_=xkT[:, :])
            wq_sb = cpool.tile([E, 48], f32r, name="wq_sb")
            nc.sync.dma_start(out=wq_sb[:, :], in_=wq[:, :])
            wk_sb = cpool.tile([E, 48], f32r, name="wk_sb")
            nc.sync.dma_start(out=wk_sb[:, :], in_=wk[:, :])
            wv_sb = cpool.tile([E, 48], f32r, name="wv_sb")
            nc.sync.dma_start(out=wv_sb[:, :], in_=wv[:, :])
            ident_sb = cpool.tile([E, E], fp32, name="ident_sb")
            nc.sync.dma_start(out=ident_sb[:, :], in_=ident[:, :])

            # ---------------- persistent work tensors ----------------
            qt = [wpool.tile([18, S], f32r, name=f"qt{h}") for h in range(2)]
            kt = [wpool.tile([18, NK], f32r, name=f"kt{h}") for h in range(2)]
            qtr = [wpool.tile([17, S], f32rr, name=f"qtr{h}") for h in range(2)]
            ktr = [wpool.tile([17, NK], f32rr, name=f"ktr{h}") for h in range(2)]
            vv = [wpool.tile([128, NKB, HD + 1], f32r, name=f"vv{h}") for h in range(2)]
            negp = [wpool.tile([128, 3 * NQB], fp32, name=f"negp{h}") for h in range(2)]
            negc = [wpool.tile([128, NQB], fp32, name=f"negc{h}") for h in range(2)]
            nT_sb = [wpool.tile([NQB, 128], f32r, name=f"nT_sb{h}") for h in range(2)]
            ctxl = wpool.tile([49, S], fp32, name="ctxl")
            ldual = wpool.tile([33, S], fp32, name="ldual")
            linv = wpool.tile([33, S], fp32, name="linv")
            lbc = wpool.tile([48, S], fp32, name="lbc")
            out_sb = wpool.tile([64, S], fp32, name="out_sb")

            # ---------------- projections: QT, KT, V ----------------
            for half in range(2):
                qt_ps = stpool.tile([48, 1024], fp32, name="qt_ps", tag="st")
                for c in range(2):
                    nc.tensor.matmul(
                        qt_ps[:, 512 * c : 512 * (c + 1)],
                        lhsT=wq_sb[:, :],
                        rhs=xT_sb[:, 1024 * half + 512 * c : 1024 * half + 512 * (c + 1)],
                        start=True,
                        stop=True,
                    )
                for h in range(2):
                    # QT rows scaled by 1/sqrt(hd)=0.25; ones row below
                    nc.scalar.mul(
                        qt[h][0:16, 1024 * half : 1024 * (half + 1)],
                        qt_ps[32 * h : 32 * h + 16, :],
                        0.25,
                    )
                    nc.scalar.mul(
                        qtr[h][0:16, 1024 * half : 1024 * (half + 1)],
                        qt_ps[32 * h : 32 * h + 16, :],
                        0.25,
                    )
            for h in range(2):
                nc.sync.dma_start(out=qt[h][16:17, :], in_=onesrow[:, :])
                nc.sync.dma_start(out=qtr[h][16:17, :], in_=onesr[:, :])

            for o, n in ((0, 512), (512, 512), (1024, 256)):
                kt_ps = apool.tile([48, 512], fp32, name="kt_ps", tag="a")
                nc.tensor.matmul(
                    kt_ps[:, 0:n],
                    lhsT=wk_sb[:, :],
                    rhs=xkT_sb[:, o : o + n],
                    start=True,
                    stop=True,
                )
                for h in range(2):
                    nc.scalar.copy(
                        kt[h][0:16, o : o + n], kt_ps[32 * h : 32 * h + 16, 0:n]
                    )
                    nc.scalar.copy(
                        ktr[h][0:16, o : o + n], kt_ps[32 * h : 32 * h + 16, 0:n]
                    )
            for h in range(2):
                nc.sync.dma_start(out=kt[h][16:17, :], in_=maskrow[:, :])
                nc.sync.dma_start(out=ktr[h][16:17, :], in_=maskr[:, :])
                nc.sync.dma_start(out=kt[h][17:18, :], in_=onesrow[:, 0:NK])
                nc.sync.dma_start(
                    out=vv[h][:, :, HD : HD + 1],
                    in_=onesrow[0:1, 0:NKB].to_broadcast([128, NKB]),
                )

            # ---------------- phase helpers ----------------
            CH = ((0, 512), (512, 512), (1024, 256))  # pass-A k chunks

            def v_iter(kb):
                v_ps = apool.tile([128, 48], fp32, name="v_ps", tag="a")
                nc.tensor.matmul(
                    v_ps[:, :],
                    lhsT=xkT_sb[:, 128 * kb : 128 * (kb + 1)],
                    rhs=wv_sb[:, :],
                    start=True,
                    stop=True,
                )
                nc.vector.tensor_copy(
                    out=vv[0][:, kb, 0:HD], in_=v_ps[:, 0:16]
                )
                nc.vector.tensor_copy(
                    out=vv[1][:, kb, 0:HD], in_=v_ps[:, 32:48]
                )

            def a_iter(h, qb):
                lhs = qtr[h][0:17, 128 * qb : 128 * (qb + 1)]
                for ci, (o, n) in enumerate(CH):
                    sc = apool.tile([128, 512], fp32, name="sc", tag="a")
                    nc.tensor.matmul(
                        sc[:, 0:n],
                        lhsT=lhs,
                        rhs=ktr[h][0:17, o : o + n],
                        start=True,
                        stop=True,
                    )
                    nc.vector.tensor_reduce(
                        negp[h][:, 3 * qb + ci : 3 * qb + ci + 1],
                        sc[:, 0:n],
                        axis=AX.X,
                        op=ALU.max,
                        negate=True,
                    )

            def negm_assemble(h):
                nc.vector.tensor_reduce(
                    negc[h][:, :],
                    negp[h].rearrange("p (b t) -> p b t", t=3),
                    axis=AX.X,
                    op=ALU.min,
                )
                ntp = apool.tile([NQB, 128], fp32, name="ntp", tag="a")
                nc.tensor.transpose(ntp[:, :], negc[h][:, :], ident_sb[:, :])
                nc.vector.tensor_copy(out=nT_sb[h][:, :], in_=ntp[:, :])
                nc.sync.dma_start(
                    out=qt[h][17:18, :].rearrange("a (b f) -> a b f", b=NQB),
                    in_=nT_sb[h][:, :],
                )

            def b_iter(h, qh, kb, ctxc):
                st = stpool.tile([128, 1024], fp32, name="st", tag="st")
                lhs = kt[h][:, 128 * kb : 128 * (kb + 1)]
                for c in range(2):
                    nc.tensor.matmul(
                        st[:, 512 * c : 512 * (c + 1)],
                        lhsT=lhs,
                        rhs=qt[h][:, 1024 * qh + 512 * c : 1024 * qh + 512 * (c + 1)],
                        start=True,
                        stop=True,
                    )
                pt = ptpool.tile([128, 1024], f32r, name="pt", tag="pt")
                nc.scalar.activation(pt[:, :], st[:, :], AF.Exp)
                for c in range(2):
                    nc.tensor.matmul(
                        ctxc[c][0:17, :],
                        lhsT=vv[h][:, kb, :],
                        rhs=pt[:, 512 * c : 512 * (c + 1)],
                        start=(kb == 0),
                        stop=(kb == NKB - 1),
                    )

            def evac(h, qh, ctxc):
                for c in range(2):
                    nc.scalar.copy(
                        ctxl[
                            32 * h : 32 * h + 17,
                            1024 * qh + 512 * c : 1024 * qh + 512 * (c + 1),
                        ],
                        ctxc[c][0:17, :],
                    )

            def b_half(h, qh):
                ctxc = [
                    ctxpool.tile([17, 512], fp32, name=f"ctx{c}", tag="ctx")
                    for c in range(2)
                ]
                return ctxc

            # ---------------- schedule ----------------
            # A(h0), with V projections interleaved
            for qb in range(NQB):
                a_iter(0, qb)
                if qb < NKB:
                    v_iter(qb)
            negm_assemble(0)

            # B(h0) (2 q-halves x NKB) overlapped with A(h1)
            ai = 0
            for qh in range(2):
                ctxc = b_half(0, qh)
                for kb in range(NKB):
                    b_iter(0, qh, kb, ctxc)
                    if ai < NQB and (kb % 2 == 0 or qh == 1):
                        a_iter(1, ai)
                        ai += 1
                evac(0, qh, ctxc)
            while ai < NQB:
                a_iter(1, ai)
                ai += 1
            negm_assemble(1)

            # B(h1)
            for qh in range(2):
                ctxc = b_half(1, qh)
                for kb in range(NKB):
                    b_iter(1, qh, kb, ctxc)
                evac(1, qh, ctxc)

            # ---------------- finals ----------------
            for h in range(2):
                nc.sync.dma_start(
                    out=ldual[32 * h : 32 * h + 1, :],
                    in_=ctxl[32 * h + 16 : 32 * h + 17, :],
                )
                nc.vector.reciprocal(
                    linv[32 * h : 32 * h + 1, :], ldual[32 * h : 32 * h + 1, :]
                )
                nc.sync.dma_start(
                    out=ldram[h : h + 1, :], in_=linv[32 * h : 32 * h + 1, :]
                )
                nc.sync.dma_start(
                    out=lbc[32 * h : 32 * h + 16, :],
                    in_=ldram[h : h + 1, :].to_broadcast([HD, S]),
                )
                nc.vector.tensor_tensor(
                    out=out_sb[32 * h : 32 * h + 16, :],
                    in0=ctxl[32 * h : 32 * h + 16, :],
                    in1=lbc[32 * h : 32 * h + 16, :],
                    op=mybir.AluOpType.mult,
                )
            for h in range(2):
                nc.sync.dma_start(
                    out=out_d[16 * h : 16 * h + 16, :],
                    in_=out_sb[32 * h : 32 * h + 16, :],
                )

    nc.finalize()
    return nc


def _prep_core_inputs(x, msk_add_full, w_query, w_key, w_value):
    """Build the 8 per-core input maps from full inputs."""
    B = x.shape[0]
    in_maps = []
    onesrow = np.ones((1, S), dtype=np.float32)
    identm = np.eye(E, dtype=np.float32)
    per_batch = []
    for b in range(B):
        keep = np.flatnonzero(msk_add_full[b] == 0.0)
        nk = len(keep)
        assert 0 < nk <= NK, f"compacted key count {nk} out of range"
        xk = np.zeros((NK, E), dtype=np.float32)
        xk[:nk] = x[b][keep]
        maskrow = np.full((1, NK), NEG, dtype=np.float32)
        maskrow[0, :nk] = 0.0
        xTb = np.ascontiguousarray(x[b].T)
        xkTb = np.ascontiguousarray(xk.T)
        per_batch.append((xTb, xkTb, maskrow))
    for c in range(8):
        b = c // 4
        h0 = 2 * (c % 4)
        xTb, xkTb, maskrow = per_batch[b]
        def _pad48(w):
            wc = np.zeros((E, 48), dtype=np.float32)
            wc[:, 0:16] = w[:, h0::8]
            wc[:, 32:48] = w[:, h0 + 1 :: 8]
            return wc

        wq_c = _pad48(w_query)
        wk_c = _pad48(w_key)
        wv_c = _pad48(w_value)
        in_maps.append(
            {
                "xT": xTb,
                "xkT": xkTb,
                "wq": wq_c,
                "wk": wk_c,
                "wv": wv_c,
                "maskrow": maskrow,
                "maskr": maskrow,
                "onesrow": onesrow,
                "onesr": onesrow,
                "ident": identm,
            }
        )
    return in_maps


def kernel(
    input_embeddings,
    token_attention_masks_source,
    token_attention_masks_target,
    masked,
    w_query,
    w_key,
    w_value,
):
    global _PROG
    x = np.asarray(input_embeddings, dtype=np.float32)
    msk = np.asarray(token_attention_masks_source)
    wq_f = np.asarray(w_query, dtype=np.float32)
    wk_f = np.asarray(w_key, dtype=np.float32)
    wv_f = np.asarray(w_value, dtype=np.float32)
    assert int(np.asarray(masked)) == 0, "only the encoder (masked=0) path is supported"
    B = x.shape[0]
    assert x.shape == (2, S, E)

    msk_add = np.where(msk == 0, np.float32(NEG), np.float32(0.0))
    in_maps = _prep_core_inputs(x, msk_add, wq_f, wk_f, wv_f)

    if _PROG is None:
        _PROG = _build_program()
    nc = _PROG

    from concourse.bass_utils import run_bass_kernel_spmd

    res = run_bass_kernel_spmd(nc, in_maps, list(range(8)))

    out = np.empty((B, S, E), dtype=np.float32)
    for c in range(8):
        b = c // 4
        h0 = 2 * (c % 4)
        o = res.results[c]["out"]  # [32, 2048]
        out[b][:, h0::8] = o[0:16, :].T
        out[b][:, h0 + 1 :: 8] = o[16:32, :].T
    return out



# revision 4
# speedup vs baseline: 1.7507x; 1.7507x over previous
"""Multi-head attention (B=2, H=8, S=2048, hd=16) on 8 Trainium2 NeuronCores.

Sharding: 16 (batch, head) groups -> 2 heads per core (cores 0-3: batch 0,
cores 4-7: batch 1).  Each core gets transposed embeddings, a key-compacted
copy (keys with source-mask 0 dropped; padded to NK with -1000 mask columns),
and the 32 projection-weight columns for its two heads.

Score matmuls run in float32r (1 cycle/row on the PE vs 4 for fp32) with
fp32-level accuracy recovered via split-precision row packing: K and Q are
each split into bf16-high + fp32-residual parts (Kh+Kl, Qh+Ql) and the four
cross products are packed into one 128-row contraction
  rows  0:16  Kh x Qh        rows 32:48  Kl x Qh(dup)
  rows 64:80  Kh(dup) x Ql   rows 96:112 Kl(dup) x Ql(dup)
  row 16: mask x ones        row 17: ones x (-rowmax)      (gaps zeroed)
Extra contraction rows are free (matmul cost is N output columns only), and
bf16-grid values pass through the PE's f32r truncation unchanged, so the sum
reconstructs the exact fp32 product.

Row-max for the softmax comes from a cheap pass over the first 512 compacted
keys only (f32r, bf16-grade): a lower-bound max within ~80 of the true max
is sufficient for fp32 exp range safety, and P(gap > 80) ~ e^-56.

ctx = P^T @ [V | 1] accumulates in PSUM with f32r operands; the ones column
gives the softmax denominator l; final scale by 1/l via DRAM-broadcast + DVE
multiply.  Output per core is [32, 2048] (dim-major); the host scatters back
into the interleaved head layout.
"""

import numpy as np

S = 2048
E = 128
HD = 16
NQB = S // 128       # 16 query blocks
NEG = -1000.0
NA = 512             # keys sampled for the row-max pass

_PROGS = {}


def _build_program(NKB):
    import concourse.mybir as mybir
    from concourse import bacc
    from concourse.tile import TileContext

    NK = 128 * NKB

    fp32 = mybir.dt.float32
    f32r = mybir.dt.float32r
    bf16 = mybir.dt.bfloat16
    AF = mybir.ActivationFunctionType
    ALU = mybir.AluOpType
    AX = mybir.AxisListType

    nc = bacc.Bacc()

    xT = nc.declare_dram_parameter("xT", [E, S], fp32, isOutput=False)
    xkT = nc.declare_dram_parameter("xkT", [E, NK], fp32, isOutput=False)
    # weight columns padded to 48: head0 dims at 0:16, head1 dims at 32:48
    wq = nc.declare_dram_parameter("wq", [E, 48], fp32, isOutput=False)
    wk = nc.declare_dram_parameter("wk", [E, 48], fp32, isOutput=False)
    wv = nc.declare_dram_parameter("wv", [E, 48], fp32, isOutput=False)
    maskrow = nc.declare_dram_parameter("maskrow", [1, NK], f32r, isOutput=False)
    onesrow = nc.declare_dram_parameter("onesrow", [1, S], f32r, isOutput=False)
    zrow = nc.declare_dram_parameter("zrow", [1, S], f32r, isOutput=False)
    ident = nc.declare_dram_parameter("ident", [E, E], fp32, isOutput=False)
    out_d = nc.declare_dram_parameter("out", [2 * HD, S], fp32, isOutput=True)
    ldram = nc.dram_tensor("ldram", [2, S], fp32)

    with TileContext(nc) as tc:
        with (
            tc.tile_pool(name="consts", bufs=1) as cpool,
            tc.tile_pool(name="work", bufs=1) as wpool,
            tc.tile_pool(name="ptp", bufs=3) as ptpool,
            tc.tile_pool(name="stp", bufs=2, space="PSUM") as stpool,
            tc.tile_pool(name="ap", bufs=2, space="PSUM") as apool,
            tc.tile_pool(name="ctxp", bufs=2, space="PSUM") as ctxpool,
        ):
            # ---------------- constant loads ----------------
            xT_sb = cpool.tile([E, S], fp32, name="xT_sb")
            nc.sync.dma_start(out=xT_sb[:, :], in_=xT[:, :])
            xkT_sb = cpool.tile([E, NK], fp32, name="xkT_sb")
            nc.sync.dma_start(out=xkT_sb[:, :], in_=xkT[:, :])
            wq_sb = cpool.tile([E, 48], fp32, name="wq_sb")
            nc.sync.dma_start(out=wq_sb[:, :], in_=wq[:, :])
            wk_sb = cpool.tile([E, 48], fp32, name="wk_sb")
            nc.sync.dma_start(out=wk_sb[:, :], in_=wk[:, :])
            wv_sb = cpool.tile([E, 48], fp32, name="wv_sb")
            nc.sync.dma_start(out=wv_sb[:, :], in_=wv[:, :])
            ident_sb = cpool.tile([E, E], fp32, name="ident_sb")
            nc.sync.dma_start(out=ident_sb[:, :], in_=ident[:, :])

            # ---------------- persistent work tensors ----------------
            qt = [wpool.tile([128, S], f32r, name=f"qt{h}") for h in range(2)]
            kt = [wpool.tile([128, NK], f32r, name=f"kt{h}") for h in range(2)]
            qhb = [wpool.tile([HD, S], bf16, name=f"qhb{h}") for h in range(2)]
            khb = [wpool.tile([HD, NK], bf16, name=f"khb{h}") for h in range(2)]
            vv = [wpool.tile([128, NKB, HD + 1], f32r, name=f"vv{h}") for h in range(2)]
            negp = [wpool.tile([128, NQB], fp32, name=f"negp{h}") for h in range(2)]
            nT_sb = [wpool.tile([NQB, 128], f32r, name=f"nT_sb{h}") for h in range(2)]
            ctxl = wpool.tile([49, S], fp32, name="ctxl")
            ldual = wpool.tile([33, S], fp32, name="ldual")
            linv = wpool.tile([33, S], fp32, name="linv")
            lbc = wpool.tile([48, S], fp32, name="lbc")
            out_sb = wpool.tile([64, S], fp32, name="out_sb")

            # zero the unused contraction rows (both sides: 0 * 0 avoids NaN
            # from stale SBUF); special rows 16/17 are overwritten below.
            for h in range(2):
                for lo, hi in ((16, 32), (48, 64), (80, 96), (112, 128)):
                    nc.sync.dma_start(
                        out=qt[h][lo:hi, :],
                        in_=zrow[0:1, 0:S].to_broadcast([hi - lo, S]),
                    )
                    nc.sync.dma_start(
                        out=kt[h][lo:hi, :],
                        in_=zrow[0:1, 0:NK].to_broadcast([hi - lo, NK]),
                    )
                nc.sync.dma_start(out=qt[h][16:17, :], in_=onesrow[:, :])
                nc.sync.dma_start(out=kt[h][16:17, :], in_=maskrow[:, :])
                nc.sync.dma_start(out=kt[h][17:18, :], in_=onesrow[:, 0:NK])
                nc.sync.dma_start(
                    out=vv[h][:, :, HD : HD + 1],
                    in_=onesrow[0:1, 0:NKB].to_broadcast([128, NKB]),
                )

            # ---------------- projections + splits ----------------
            # Q: 1/sqrt(hd) folded into wq host-side.  Per 1024-col half:
            for half in range(2):
                cs = slice(1024 * half, 1024 * (half + 1))
                qt_ps = stpool.tile([48, 1024], fp32, name="qt_ps", tag="st")
                for c in range(2):
                    nc.tensor.matmul(
                        qt_ps[:, 512 * c : 512 * (c + 1)],
                        lhsT=wq_sb[:, :],
                        rhs=xT_sb[:, 1024 * half + 512 * c : 1024 * half + 512 * (c + 1)],
                        start=True,
                        stop=True,
                    )
                for h in range(2):
                    ps = qt_ps[32 * h : 32 * h + 16, :]
                    nc.scalar.copy(qhb[h][:, cs], ps)                     # ACT: bf16 round
                    nc.vector.tensor_copy(out=qt[h][0:16, cs], in_=qhb[h][:, cs])
                    nc.vector.tensor_tensor(
                        out=qt[h][64:80, cs], in0=ps, in1=qt[h][0:16, cs], op=ALU.subtract
                    )
                    nc.sync.dma_start(out=qt[h][32:48, cs], in_=qt[h][0:16, cs])
                    nc.sync.dma_start(out=qt[h][96:112, cs], in_=qt[h][64:80, cs])

            CH = tuple(
                (o, min(512, NK - o)) for o in range(0, NK, 512)
            )  # K-proj chunks
            for o, n in CH:
                cs = slice(o, o + n)
                kt_ps = stpool.tile([48, 512], fp32, name="kt_ps", tag="st")
                nc.tensor.matmul(
                    kt_ps[:, 0:n],
                    lhsT=wk_sb[:, :],
                    rhs=xkT_sb[:, cs],
                    start=True,
                    stop=True,
                )
                for h in range(2):
                    ps = kt_ps[32 * h : 32 * h + 16, 0:n]
                    nc.scalar.copy(khb[h][:, cs], ps)                     # ACT: bf16 round
                    nc.vector.tensor_copy(out=kt[h][0:16, cs], in_=khb[h][:, cs])
                    nc.vector.tensor_tensor(
                        out=kt[h][32:48, cs], in0=ps, in1=kt[h][0:16, cs], op=ALU.subtract
                    )
                    nc.sync.dma_start(out=kt[h][64:80, cs], in_=kt[h][0:16, cs])
                    nc.sync.dma_start(out=kt[h][96:112, cs], in_=kt[h][32:48, cs])

            def v_iter(kb):
                v_ps = apool.tile([128, 48], fp32, name="v_ps", tag="a")
                nc.tensor.matmul(
                    v_ps[:, :],
                    lhsT=xkT_sb[:, 128 * kb : 128 * (kb + 1)],
                    rhs=wv_sb[:, :],
                    start=True,
                    stop=True,
                )
                nc.vector.tensor_copy(out=vv[0][:, kb, 0:HD], in_=v_ps[:, 0:16])
                nc.vector.tensor_copy(out=vv[1][:, kb, 0:HD], in_=v_ps[:, 32:48])

            # ---------------- pass A: subsampled row-max ----------------
            def a_iter(h, qb):
                sc = apool.tile([128, NA], fp32, name="sc", tag="a")
                nc.tensor.matmul(
                    sc[:, :],
                    lhsT=qt[h][0:17, 128 * qb : 128 * (qb + 1)],
                    rhs=kt[h][0:17, 0:NA],
                    start=True,
                    stop=True,
                )
                nc.vector.tensor_reduce(
                    negp[h][:, qb : qb + 1],
                    sc[:, :],
                    axis=AX.X,
                    op=ALU.max,
                    negate=True,
                )

            def negm_assemble(h):
                ntp = apool.tile([NQB, 128], fp32, name="ntp", tag="a")
                nc.tensor.transpose(ntp[:, :], negp[h][:, :], ident_sb[:, :])
                nc.vector.tensor_copy(out=nT_sb[h][:, :], in_=ntp[:, :])
                nc.sync.dma_start(
                    out=qt[h][17:18, :].rearrange("a (b f) -> a b f", b=NQB),
                    in_=nT_sb[h][:, :],
                )

            # ---------------- pass B + ctx ----------------
            def b_iter(h, qh, kb, ctxc):
                st = stpool.tile([128, 1024], fp32, name="st", tag="st")
                lhs = kt[h][:, 128 * kb : 128 * (kb + 1)]
                for c in range(2):
                    nc.tensor.matmul(
                        st[:, 512 * c : 512 * (c + 1)],
                        lhsT=lhs,
                        rhs=qt[h][:, 1024 * qh + 512 * c : 1024 * qh + 512 * (c + 1)],
                        start=True,
                        stop=True,
                    )
                pt = ptpool.tile([128, 1024], f32r, name="pt", tag="pt")
                nc.scalar.activation(pt[:, :], st[:, :], AF.Exp)
                for c in range(2):
                    nc.tensor.matmul(
                        ctxc[c][0:17, :],
                        lhsT=vv[h][:, kb, :],
                        rhs=pt[:, 512 * c : 512 * (c + 1)],
                        start=(kb == 0),
                        stop=(kb == NKB - 1),
                    )

            def evac(h, qh, ctxc):
                for c in range(2):
                    nc.scalar.copy(
                        ctxl[
                            32 * h : 32 * h + 17,
                            1024 * qh + 512 * c : 1024 * qh + 512 * (c + 1),
                        ],
                        ctxc[c][0:17, :],
                    )

            def b_half(h, qh):
                return [
                    ctxpool.tile([17, 512], fp32, name=f"ctx{c}", tag="ctx")
                    for c in range(2)
                ]

            # ---------------- schedule ----------------
            for qb in range(NQB):
                a_iter(0, qb)
                if qb < NKB:
                    v_iter(qb)
            negm_assemble(0)

            ai = 0
            for qh in range(2):
                ctxc = b_half(0, qh)
                for kb in range(NKB):
                    b_iter(0, qh, kb, ctxc)
                    if ai < NQB and (kb % 2 == 0 or qh == 1):
                        a_iter(1, ai)
                        ai += 1
                evac(0, qh, ctxc)
            while ai < NQB:
                a_iter(1, ai)
                ai += 1
            negm_assemble(1)

            for qh in range(2):
                ctxc = b_half(1, qh)
                for kb in range(NKB):
                    b_iter(1, qh, kb, ctxc)
                evac(1, qh, ctxc)

            # ---------------- finals ----------------
            for h in range(2):
                nc.sync.dma_start(
                    out=ldual[32 * h : 32 * h + 1, :],
                    in_=ctxl[32 * h + 16 : 32 * h + 17, :],
                )
                nc.vector.reciprocal(
                    linv[32 * h : 32 * h + 1, :], ldual[32 * h : 32 * h + 1, :]
                )
                nc.sync.dma_start(
                    out=ldram[h : h + 1, :], in_=linv[32 * h : 32 * h + 1, :]
                )
                nc.sync.dma_start(
                    out=lbc[32 * h : 32 * h + 16, :],
                    in_=ldram[h : h + 1, :].to_broadcast([HD, S]),
                )
                nc.vector.tensor_tensor(
                    out=out_sb[32 * h : 32 * h + 16, :],
                    in0=ctxl[32 * h : 32 * h + 16, :],
                    in1=lbc[32 * h : 32 * h + 16, :],
                    op=mybir.AluOpType.mult,
                )
            for h in range(2):
                nc.sync.dma_start(
                    out=out_d[16 * h : 16 * h + 16, :],
                    in_=out_sb[32 * h : 32 * h + 16, :],
                )

    nc.finalize()
    return nc


def _prep_core_inputs(x, msk_add_full, w_query, w_key, w_value):
    """Build the 8 per-core input maps from full inputs.  Returns (maps, NKB)."""
    B = x.shape[0]
    onesrow = np.ones((1, S), dtype=np.float32)
    zrow = np.zeros((1, S), dtype=np.float32)
    identm = np.eye(E, dtype=np.float32)

    keeps = [np.flatnonzero(msk_add_full[b] == 0.0) for b in range(B)]
    max_nk = max(len(k) for k in keeps)
    assert max_nk >= NA, "row-max subsample needs >= NA valid keys"
    NKB = -(-max_nk // 128)  # ceil to 128
    NK = 128 * NKB

    per_batch = []
    for b in range(B):
        keep = keeps[b]
        nk = len(keep)
        xk = np.zeros((NK, E), dtype=np.float32)
        xk[:nk] = x[b][keep]
        maskrow = np.full((1, NK), NEG, dtype=np.float32)
        maskrow[0, :nk] = 0.0
        xTb = np.ascontiguousarray(x[b].T)
        xkTb = np.ascontiguousarray(xk.T)
        per_batch.append((xTb, xkTb, maskrow))

    in_maps = []
    for c in range(8):
        b = c // 4
        h0 = 2 * (c % 4)
        xTb, xkTb, maskrow = per_batch[b]

        def _pad48(w, scale=1.0):
            wc = np.zeros((E, 48), dtype=np.float32)
            wc[:, 0:16] = w[:, h0::8] * scale
            wc[:, 32:48] = w[:, h0 + 1 :: 8] * scale
            return wc

        in_maps.append(
            {
                "xT": xTb,
                "xkT": xkTb,
                "wq": _pad48(w_query, 0.25),  # 1/sqrt(hd) folded in (exact)
                "wk": _pad48(w_key),
                "wv": _pad48(w_value),
                "maskrow": maskrow,
                "onesrow": onesrow,
                "zrow": zrow,
                "ident": identm,
            }
        )
    return in_maps, NKB


def kernel(
    input_embeddings,
    token_attention_masks_source,
    token_attention_masks_target,
    masked,
    w_query,
    w_key,
    w_value,
):
    x = np.asarray(input_embeddings, dtype=np.float32)
    msk = np.asarray(token_attention_masks_source)
    wq_f = np.asarray(w_query, dtype=np.float32)
    wk_f = np.asarray(w_key, dtype=np.float32)
    wv_f = np.asarray(w_value, dtype=np.float32)
    assert int(np.asarray(masked)) == 0, "only the encoder (masked=0) path is supported"
    B = x.shape[0]
    assert x.shape == (2, S, E)

    msk_add = np.where(msk == 0, np.float32(NEG), np.float32(0.0))
    in_maps, NKB = _prep_core_inputs(x, msk_add, wq_f, wk_f, wv_f)

    if NKB not in _PROGS:
        _PROGS[NKB] = _build_program(NKB)
    nc = _PROGS[NKB]
    global _PROG
    _PROG = nc

    from concourse.bass_utils import run_bass_kernel_spmd

    res = run_bass_kernel_spmd(nc, in_maps, list(range(8)))

    out = np.empty((B, S, E), dtype=np.float32)
    for c in range(8):
        b = c // 4
        h0 = 2 * (c % 4)
        o = res.results[c]["out"]  # [32, 2048]
        out[b][:, h0::8] = o[0:16, :].T
        out[b][:, h0 + 1 :: 8] = o[16:32, :].T

    # The device row-max is a lower bound from a 512-key subsample; rows where
    # the true max exceeds it by >~88 overflow exp to inf (-> inf or NaN or,
    # when only the denominator overflows, an exact-zero vector).  Those rows
    # are deterministic and rare (<1%); recompute them exactly on host.
    for b in range(B):
        for h in range(8):
            hv = out[b][:, h::8]  # [S, 16]
            bad = ~np.isfinite(hv).all(axis=1) | (hv == 0.0).all(axis=1)
            if not bad.any():
                continue
            rows = np.flatnonzero(bad)
            xb = x[b].astype(np.float64)
            qh = (xb[rows] @ wq_f[:, h::8].astype(np.float64)) * 0.25
            kh = xb @ wk_f[:, h::8].astype(np.float64)
            vh = xb @ wv_f[:, h::8].astype(np.float64)
            sc = qh @ kh.T + msk_add[b][None, :].astype(np.float64)
            sc -= sc.max(axis=1, keepdims=True)
            p = np.exp(sc)
            p /= p.sum(axis=1, keepdims=True)
            out[b][rows, h::8] = (p @ vh).astype(np.float32)
    return out


_PROG = None


# revision 6
# speedup vs baseline: 2.2267x; 1.2719x over previous
"""Multi-head attention (B=2, H=8, S=2048, hd=16) on 8 Trainium2 NeuronCores.

Sharding: 16 (batch, head) groups -> 2 heads per core (cores 0-3: batch 0,
cores 4-7: batch 1).  Each core gets transposed embeddings, a key-compacted
copy (keys with source-mask 0 dropped; padded to NK with -1000 mask columns),
and the 32 projection-weight columns for its two heads.

Score matmuls run in float32r (1 cycle/row on the PE vs 4 for fp32) with
fp32-level accuracy recovered via split-precision row packing: K and Q are
each split into bf16-high + fp32-residual parts (Kh+Kl, Qh+Ql) and the four
cross products are packed into one 128-row contraction
  rows  0:16  Kh x Qh        rows 32:48  Kl x Qh(dup)
  rows 64:80  Kh(dup) x Ql   rows 96:112 Kl(dup) x Ql(dup)
  row 16: mask x ones        row 17: ones x (-rowmax)      (gaps zeroed)
Extra contraction rows are free (matmul cost is N output columns only), and
bf16-grid values pass through the PE's f32r truncation unchanged, so the sum
reconstructs the exact fp32 product.

Row-max comes from a cheap pass over the first NA=512 compacted keys (f32r,
bf16-grade): a lower-bound max keeps exp finite unless the excluded keys beat
the subsample by >88 (logistic tail, <1% of rows); those rows come back as
inf/NaN/zero and are recomputed exactly on the host.

ctx = P^T @ [V | 1] accumulates in PSUM with f32r operands; the ones column
gives the softmax denominator l; 1/l is computed on a [128,16] reshape (not
the serial [1,2048] row) and applied via DRAM-broadcast + DVE multiply.
Output per core is [32, 2048] (dim-major); the host scatters back into the
interleaved head layout.
"""

import numpy as np

S = 2048
E = 128
HD = 16
NQB = S // 128       # 16 query blocks
NEG = -1000.0
NA = 512             # keys sampled for the row-max pass

_PROGS = {}


def _build_program(NKB):
    import concourse.mybir as mybir
    from concourse import bacc
    from concourse.tile import TileContext

    NK = 128 * NKB

    fp32 = mybir.dt.float32
    f32r = mybir.dt.float32r
    bf16 = mybir.dt.bfloat16
    AF = mybir.ActivationFunctionType
    ALU = mybir.AluOpType
    AX = mybir.AxisListType

    nc = bacc.Bacc()

    xT = nc.declare_dram_parameter("xT", [E, S], fp32, isOutput=False)
    xkT = nc.declare_dram_parameter("xkT", [E, NK], fp32, isOutput=False)
    # weight columns padded to 48: head0 dims at 0:16, head1 dims at 32:48
    wq = nc.declare_dram_parameter("wq", [E, 48], fp32, isOutput=False)
    wk = nc.declare_dram_parameter("wk", [E, 48], fp32, isOutput=False)
    wv = nc.declare_dram_parameter("wv", [E, 48], fp32, isOutput=False)
    maskrow = nc.declare_dram_parameter("maskrow", [1, NK], f32r, isOutput=False)
    onesrow = nc.declare_dram_parameter("onesrow", [1, S], f32r, isOutput=False)
    zrow = nc.declare_dram_parameter("zrow", [1, S], f32r, isOutput=False)
    ident = nc.declare_dram_parameter("ident", [E, E], fp32, isOutput=False)
    out_d = nc.declare_dram_parameter("out", [2 * HD, S], fp32, isOutput=True)
    ldram = nc.dram_tensor("ldram", [2, S], fp32)

    with TileContext(nc) as tc:
        with (
            tc.tile_pool(name="consts", bufs=1) as cpool,
            tc.tile_pool(name="work", bufs=1) as wpool,
            tc.tile_pool(name="ptp", bufs=3) as ptpool,
            tc.tile_pool(name="stp", bufs=2, space="PSUM") as stpool,
            tc.tile_pool(name="ap", bufs=2, space="PSUM") as apool,
            tc.tile_pool(name="ctxp", bufs=2, space="PSUM") as ctxpool,
        ):
            # ---------------- input loads first (sync-queue order) ----------
            xT_sb = cpool.tile([E, S], fp32, name="xT_sb")
            nc.sync.dma_start(out=xT_sb[:, :], in_=xT[:, :])
            wq_sb = cpool.tile([E, 48], fp32, name="wq_sb")
            nc.sync.dma_start(out=wq_sb[:, :], in_=wq[:, :])
            wk_sb = cpool.tile([E, 48], fp32, name="wk_sb")
            nc.sync.dma_start(out=wk_sb[:, :], in_=wk[:, :])
            wv_sb = cpool.tile([E, 48], fp32, name="wv_sb")
            nc.sync.dma_start(out=wv_sb[:, :], in_=wv[:, :])
            xkT_sb = cpool.tile([E, NK], fp32, name="xkT_sb")
            nc.sync.dma_start(out=xkT_sb[:, :], in_=xkT[:, :])
            ident_sb = cpool.tile([E, E], fp32, name="ident_sb")
            nc.sync.dma_start(out=ident_sb[:, :], in_=ident[:, :])

            # ---------------- persistent work tensors ----------------
            qt = [wpool.tile([128, S], f32r, name=f"qt{h}") for h in range(2)]
            kt = [wpool.tile([128, NK], f32r, name=f"kt{h}") for h in range(2)]
            qhb = [wpool.tile([HD, S], bf16, name=f"qhb{h}") for h in range(2)]
            khb = [wpool.tile([HD, NK], bf16, name=f"khb{h}") for h in range(2)]
            vv = [wpool.tile([128, NKB, HD + 1], f32r, name=f"vv{h}") for h in range(2)]
            negp = [wpool.tile([128, NQB], fp32, name=f"negp{h}") for h in range(2)]
            nT8 = [
                [wpool.tile([NQB // 2, 128], f32r, name=f"nT8_{h}{hf}") for hf in range(2)]
                for h in range(2)
            ]
            ctxl = wpool.tile([49, S], fp32, name="ctxl")
            lsq = wpool.tile([128, 2 * HD], fp32, name="lsq")
            lisq = wpool.tile([128, 2 * HD], fp32, name="lisq")
            lbc = wpool.tile([48, S], fp32, name="lbc")
            out_sb = wpool.tile([64, S], fp32, name="out_sb")

            # special rows + zero fill for the unused contraction rows (both
            # sides: 0 * 0 avoids NaN from stale SBUF).  Rows 32:48/64:80/
            # 96:112 inside [18:128) are re-written by the split producers
            # below; WAW deps keep the order right.
            for h in range(2):
                nc.sync.dma_start(out=qt[h][16:17, :], in_=onesrow[:, :])
                nc.sync.dma_start(out=kt[h][16:17, :], in_=maskrow[:, :])
                nc.sync.dma_start(out=kt[h][17:18, :], in_=onesrow[:, 0:NK])
                nc.sync.dma_start(
                    out=qt[h][18:128, :], in_=zrow[0:1, 0:S].to_broadcast([110, S])
                )
                nc.sync.dma_start(
                    out=kt[h][18:128, :], in_=zrow[0:1, 0:NK].to_broadcast([110, NK])
                )
                nc.sync.dma_start(
                    out=vv[h][:, :, HD : HD + 1],
                    in_=onesrow[0:1, 0:NKB].to_broadcast([128, NKB]),
                )

            # ---------------- projections + splits ----------------
            # Q: 1/sqrt(hd) folded into wq host-side.  Per 1024-col half:
            def q_proj(half):
                cs = slice(1024 * half, 1024 * (half + 1))
                qt_ps = stpool.tile([48, 1024], fp32, name="qt_ps", tag="st")
                for c in range(2):
                    nc.tensor.matmul(
                        qt_ps[:, 512 * c : 512 * (c + 1)],
                        lhsT=wq_sb[:, :],
                        rhs=xT_sb[:, 1024 * half + 512 * c : 1024 * half + 512 * (c + 1)],
                        start=True,
                        stop=True,
                    )
                for h in range(2):
                    ps = qt_ps[32 * h : 32 * h + 16, :]
                    nc.scalar.copy(qhb[h][:, cs], ps)                 # bf16 round
                    nc.vector.tensor_copy(out=qt[h][0:16, cs], in_=qhb[h][:, cs])
                    nc.vector.tensor_tensor(
                        out=qt[h][64:80, cs], in0=ps, in1=qt[h][0:16, cs], op=ALU.subtract
                    )
                    nc.sync.dma_start(out=qt[h][32:48, cs], in_=qt[h][0:16, cs])
                    nc.sync.dma_start(out=qt[h][96:112, cs], in_=qt[h][64:80, cs])

            def k_proj(o, n):
                cs = slice(o, o + n)
                kt_ps = stpool.tile([48, 512], fp32, name="kt_ps", tag="st")
                nc.tensor.matmul(
                    kt_ps[:, 0:n], lhsT=wk_sb[:, :], rhs=xkT_sb[:, cs], start=True, stop=True
                )
                for h in range(2):
                    ps = kt_ps[32 * h : 32 * h + 16, 0:n]
                    nc.scalar.copy(khb[h][:, cs], ps)                 # bf16 round
                    nc.vector.tensor_copy(out=kt[h][0:16, cs], in_=khb[h][:, cs])
                    nc.vector.tensor_tensor(
                        out=kt[h][32:48, cs], in0=ps, in1=kt[h][0:16, cs], op=ALU.subtract
                    )
                    nc.sync.dma_start(out=kt[h][64:80, cs], in_=kt[h][0:16, cs])
                    nc.sync.dma_start(out=kt[h][96:112, cs], in_=kt[h][32:48, cs])

            def v_iter(kb):
                v_ps = apool.tile([128, 48], fp32, name="v_ps", tag="a")
                nc.tensor.matmul(
                    v_ps[:, :],
                    lhsT=xkT_sb[:, 128 * kb : 128 * (kb + 1)],
                    rhs=wv_sb[:, :],
                    start=True,
                    stop=True,
                )
                nc.vector.tensor_copy(out=vv[0][:, kb, 0:HD], in_=v_ps[:, 0:16])
                nc.vector.tensor_copy(out=vv[1][:, kb, 0:HD], in_=v_ps[:, 32:48])

            # ---------------- pass A: subsampled row-max ----------------
            def a_iter(h, qb):
                sc = apool.tile([128, NA], fp32, name="sc", tag="a")
                nc.tensor.matmul(
                    sc[:, :],
                    lhsT=qt[h][0:17, 128 * qb : 128 * (qb + 1)],
                    rhs=kt[h][0:17, 0:NA],
                    start=True,
                    stop=True,
                )
                nc.vector.tensor_reduce(
                    negp[h][:, qb : qb + 1], sc[:, :], axis=AX.X, op=ALU.max, negate=True
                )

            def negm_half(h, hf):
                nq = NQB // 2
                ntp = apool.tile([nq, 128], fp32, name="ntp", tag="a")
                nc.tensor.transpose(
                    ntp[:, :], negp[h][:, nq * hf : nq * (hf + 1)], ident_sb[:, :]
                )
                nc.vector.tensor_copy(out=nT8[h][hf][:, :], in_=ntp[:, :])
                nc.sync.dma_start(
                    out=qt[h][17:18, 1024 * hf : 1024 * (hf + 1)].rearrange(
                        "a (b f) -> a b f", b=nq
                    ),
                    in_=nT8[h][hf][:, :],
                )

            # ---------------- pass B + ctx ----------------
            def b_iter(h, qh, kb, ctxc):
                st = stpool.tile([128, 1024], fp32, name="st", tag="st")
                lhs = kt[h][:, 128 * kb : 128 * (kb + 1)]
                for c in range(2):
                    nc.tensor.matmul(
                        st[:, 512 * c : 512 * (c + 1)],
                        lhsT=lhs,
                        rhs=qt[h][:, 1024 * qh + 512 * c : 1024 * qh + 512 * (c + 1)],
                        start=True,
                        stop=True,
                    )
                pt = ptpool.tile([128, 1024], f32r, name="pt", tag="pt")
                nc.scalar.activation(pt[:, :], st[:, :], AF.Exp)
                for c in range(2):
                    nc.tensor.matmul(
                        ctxc[c][0:17, :],
                        lhsT=vv[h][:, kb, :],
                        rhs=pt[:, 512 * c : 512 * (c + 1)],
                        start=(kb == 0),
                        stop=(kb == NKB - 1),
                    )

            def evac(h, qh, ctxc, last=False):
                eng = nc.vector if last else nc.scalar
                for c in range(2):
                    dst = ctxl[
                        32 * h : 32 * h + 17,
                        1024 * qh + 512 * c : 1024 * qh + 512 * (c + 1),
                    ]
                    if last:
                        nc.vector.tensor_copy(out=dst, in_=ctxc[c][0:17, :])
                    else:
                        nc.scalar.copy(dst, ctxc[c][0:17, :])

            def b_half(h, qh):
                return [
                    ctxpool.tile([17, 512], fp32, name=f"ctx{c}", tag="ctx")
                    for c in range(2)
                ]

            def finals(h):
                # l row -> [128,16] square, parallel reciprocal, back to a row
                nc.sync.dma_start(
                    out=lsq[:, 16 * h : 16 * h + 16],
                    in_=ctxl[32 * h + 16 : 32 * h + 17, :].rearrange(
                        "a (b f) -> a b f", b=128
                    ),
                )
                nc.vector.reciprocal(
                    lisq[:, 16 * h : 16 * h + 16], lsq[:, 16 * h : 16 * h + 16]
                )
                nc.sync.dma_start(
                    out=ldram[h : h + 1, :].rearrange("a (b f) -> a b f", b=128),
                    in_=lisq[:, 16 * h : 16 * h + 16],
                )
                nc.sync.dma_start(
                    out=lbc[32 * h : 32 * h + 16, :],
                    in_=ldram[h : h + 1, :].to_broadcast([HD, S]),
                )
                nc.vector.tensor_tensor(
                    out=out_sb[32 * h : 32 * h + 16, :],
                    in0=ctxl[32 * h : 32 * h + 16, :],
                    in1=lbc[32 * h : 32 * h + 16, :],
                    op=ALU.mult,
                )
                nc.sync.dma_start(
                    out=out_d[16 * h : 16 * h + 16, :],
                    in_=out_sb[32 * h : 32 * h + 16, :],
                )

            # ---------------- schedule ----------------
            q_proj(0)
            k_proj(0, 512)
            q_proj(1)
            for o, n in tuple((o, min(512, NK - o)) for o in range(512, NK, 512)):
                k_proj(o, n)

            # A(h0) first half, with V projections interleaved
            for qb in range(NQB // 2):
                a_iter(0, qb)
                v_iter(qb)
            negm_half(0, 0)

            # B(h0,qh0) | A(h0) second half + V tail
            ctxc = b_half(0, 0)
            for kb in range(NKB):
                b_iter(0, 0, kb, ctxc)
                if kb < NQB // 2:
                    a_iter(0, NQB // 2 + kb)
                if NQB // 2 + kb < NKB:
                    v_iter(NQB // 2 + kb)
            negm_half(0, 1)
            evac(0, 0, ctxc)

            # B(h0,qh1) | A(h1) first half
            ctxc = b_half(0, 1)
            for kb in range(NKB):
                b_iter(0, 1, kb, ctxc)
                if kb < NQB // 2:
                    a_iter(1, kb)
            negm_half(1, 0)
            evac(0, 1, ctxc)

            # B(h1,qh0) | A(h1) second half ; finals(h0) overlap
            ctxc = b_half(1, 0)
            for kb in range(NKB):
                b_iter(1, 0, kb, ctxc)
                if kb < NQB // 2:
                    a_iter(1, NQB // 2 + kb)
            negm_half(1, 1)
            finals(0)
            evac(1, 0, ctxc)

            # B(h1,qh1)
            ctxc = b_half(1, 1)
            for kb in range(NKB):
                b_iter(1, 1, kb, ctxc)
            evac(1, 1, ctxc, last=True)
            finals(1)

    nc.finalize()
    return nc


def _prep_core_inputs(x, msk_add_full, w_query, w_key, w_value):
    """Build the 8 per-core input maps from full inputs.  Returns (maps, NKB)."""
    B = x.shape[0]
    onesrow = np.ones((1, S), dtype=np.float32)
    zrow = np.zeros((1, S), dtype=np.float32)
    identm = np.eye(E, dtype=np.float32)

    keeps = [np.flatnonzero(msk_add_full[b] == 0.0) for b in range(B)]
    max_nk = max(len(k) for k in keeps)
    assert max_nk >= NA, "row-max subsample needs >= NA valid keys"
    NKB = -(-max_nk // 128)  # ceil to 128
    NK = 128 * NKB

    per_batch = []
    for b in range(B):
        keep = keeps[b]
        nk = len(keep)
        xk = np.zeros((NK, E), dtype=np.float32)
        xk[:nk] = x[b][keep]
        maskrow = np.full((1, NK), NEG, dtype=np.float32)
        maskrow[0, :nk] = 0.0
        xTb = np.ascontiguousarray(x[b].T)
        xkTb = np.ascontiguousarray(xk.T)
        per_batch.append((xTb, xkTb, maskrow))

    in_maps = []
    for c in range(8):
        b = c // 4
        h0 = 2 * (c % 4)
        xTb, xkTb, maskrow = per_batch[b]

        def _pad48(w, scale=1.0):
            wc = np.zeros((E, 48), dtype=np.float32)
            wc[:, 0:16] = w[:, h0::8] * scale
            wc[:, 32:48] = w[:, h0 + 1 :: 8] * scale
            return wc

        in_maps.append(
            {
                "xT": xTb,
                "xkT": xkTb,
                "wq": _pad48(w_query, 0.25),  # 1/sqrt(hd) folded in (exact)
                "wk": _pad48(w_key),
                "wv": _pad48(w_value),
                "maskrow": maskrow,
                "onesrow": onesrow,
                "zrow": zrow,
                "ident": identm,
            }
        )
    return in_maps, NKB


def kernel(
    input_embeddings,
    token_attention_masks_source,
    token_attention_masks_target,
    masked,
    w_query,
    w_key,
    w_value,
):
    x = np.asarray(input_embeddings, dtype=np.float32)
    msk = np.asarray(token_attention_masks_source)
    wq_f = np.asarray(w_query, dtype=np.float32)
    wk_f = np.asarray(w_key, dtype=np.float32)
    wv_f = np.asarray(w_value, dtype=np.float32)
    assert int(np.asarray(masked)) == 0, "only the encoder (masked=0) path is supported"
    B = x.shape[0]
    assert x.shape == (2, S, E)

    msk_add = np.where(msk == 0, np.float32(NEG), np.float32(0.0))
    in_maps, NKB = _prep_core_inputs(x, msk_add, wq_f, wk_f, wv_f)

    if NKB not in _PROGS:
        _PROGS[NKB] = _build_program(NKB)
    nc = _PROGS[NKB]
    global _PROG
    _PROG = nc

    from concourse.bass_utils import run_bass_kernel_spmd

    res = run_bass_kernel_spmd(nc, in_maps, list(range(8)))

    out = np.empty((B, S, E), dtype=np.float32)
    for c in range(8):
        b = c // 4
        h0 = 2 * (c % 4)
        o = res.results[c]["out"]  # [32, 2048]
        out[b][:, h0::8] = o[0:16, :].T
        out[b][:, h0 + 1 :: 8] = o[16:32, :].T

    # The device row-max is a lower bound from a 512-key subsample; rows where
    # the true max exceeds it by >~88 overflow exp to inf (-> inf or NaN or,
    # when only the denominator overflows, an exact-zero vector).  Those rows
    # are deterministic and rare (<1%); recompute them exactly on host.
    for b in range(B):
        for h in range(8):
            hv = out[b][:, h::8]  # [S, 16]
            bad = ~np.isfinite(hv).all(axis=1) | (hv == 0.0).all(axis=1)
            if not bad.any():
                continue
            rows = np.flatnonzero(bad)
            xb = x[b].astype(np.float64)
            qh = (xb[rows] @ wq_f[:, h::8].astype(np.float64)) * 0.25
            kh = xb @ wk_f[:, h::8].astype(np.float64)
            vh = xb @ wv_f[:, h::8].astype(np.float64)
            sc = qh @ kh.T + msk_add[b][None, :].astype(np.float64)
            sc -= sc.max(axis=1, keepdims=True)
            p = np.exp(sc)
            p /= p.sum(axis=1, keepdims=True)
            out[b][rows, h::8] = (p @ vh).astype(np.float32)
    return out


_PROG = None


# revision 7
# speedup vs baseline: 2.2771x; 1.0227x over previous
"""Multi-head attention (B=2, H=8, S=2048, hd=16) on 8 Trainium2 NeuronCores.

Sharding: 16 (batch, head) groups -> 2 heads per core (cores 0-3: batch 0,
cores 4-7: batch 1).  Each core gets transposed embeddings, a key-compacted
copy (keys with source-mask 0 dropped; padded to NK with -1000 mask columns),
and the 32 projection-weight columns for its two heads.

Score matmuls run in float32r (1 cycle/row on the PE vs 4 for fp32) with
fp32-level accuracy recovered via split-precision row packing: K and Q are
each split into bf16-high + fp32-residual parts (Kh+Kl, Qh+Ql) and the four
cross products are packed into one 128-row contraction
  rows  0:16  Kh x Qh        rows 32:48  Kl x Qh(dup)
  rows 64:80  Kh(dup) x Ql   rows 96:112 Kl(dup) x Ql(dup)
  row 16: mask x ones        row 17: ones x (-rowmax)      (gaps zeroed)
Extra contraction rows are free (matmul cost is N output columns only), and
bf16-grid values pass through the PE's f32r truncation unchanged, so the sum
reconstructs the exact fp32 product.

Row-max comes from a cheap pass over the first NA=512 compacted keys (f32r,
bf16-grade): a lower-bound max keeps exp finite unless the excluded keys beat
the subsample by >88 (logistic tail, <1% of rows); those rows come back as
inf/NaN/zero and are recomputed exactly on the host.

ctx = P^T @ [V | 1] accumulates in PSUM with f32r operands; the ones column
gives the softmax denominator l; 1/l is computed on a [128,16] reshape (not
the serial [1,2048] row) and applied via DRAM-broadcast + DVE multiply.
Output per core is [32, 2048] (dim-major); the host scatters back into the
interleaved head layout.
"""

import numpy as np

S = 2048
E = 128
HD = 16
NQB = S // 128       # 16 query blocks
NEG = -1000.0
NA = 512             # keys sampled for the row-max pass

_PROGS = {}


def _build_program(NKB):
    import concourse.mybir as mybir
    from concourse import bacc
    from concourse.tile import TileContext

    NK = 128 * NKB

    fp32 = mybir.dt.float32
    f32r = mybir.dt.float32r
    bf16 = mybir.dt.bfloat16
    AF = mybir.ActivationFunctionType
    ALU = mybir.AluOpType
    AX = mybir.AxisListType

    nc = bacc.Bacc()

    xT = nc.declare_dram_parameter("xT", [E, S], fp32, isOutput=False)
    xkT = nc.declare_dram_parameter("xkT", [E, NK], fp32, isOutput=False)
    # weight columns padded to 48: head0 dims at 0:16, head1 dims at 32:48
    wq = nc.declare_dram_parameter("wq", [E, 48], fp32, isOutput=False)
    wk = nc.declare_dram_parameter("wk", [E, 48], fp32, isOutput=False)
    wv = nc.declare_dram_parameter("wv", [E, 48], fp32, isOutput=False)
    maskrow = nc.declare_dram_parameter("maskrow", [1, NK], f32r, isOutput=False)
    onesrow = nc.declare_dram_parameter("onesrow", [1, S], f32r, isOutput=False)
    zrow = nc.declare_dram_parameter("zrow", [1, S], f32r, isOutput=False)
    ident = nc.declare_dram_parameter("ident", [E, E], fp32, isOutput=False)
    out_d = nc.declare_dram_parameter("out", [2 * HD, S], fp32, isOutput=True)
    ldram = nc.dram_tensor("ldram", [2, S], fp32)

    with TileContext(nc) as tc:
        with (
            tc.tile_pool(name="consts", bufs=1) as cpool,
            tc.tile_pool(name="work", bufs=1) as wpool,
            tc.tile_pool(name="ptp", bufs=3) as ptpool,
            tc.tile_pool(name="stp", bufs=2, space="PSUM") as stpool,
            tc.tile_pool(name="ap", bufs=2, space="PSUM") as apool,
            tc.tile_pool(name="ctxp", bufs=2, space="PSUM") as ctxpool,
        ):
            # ---------------- input loads first (sync-queue order) ----------
            xT_sb = cpool.tile([E, S], fp32, name="xT_sb")
            nc.sync.dma_start(out=xT_sb[:, 0:1024], in_=xT[:, 0:1024])
            nc.sync.dma_start(out=xT_sb[:, 1024:2048], in_=xT[:, 1024:2048])
            wq_sb = cpool.tile([E, 48], fp32, name="wq_sb")
            nc.sync.dma_start(out=wq_sb[:, :], in_=wq[:, :])
            wk_sb = cpool.tile([E, 48], fp32, name="wk_sb")
            nc.sync.dma_start(out=wk_sb[:, :], in_=wk[:, :])
            wv_sb = cpool.tile([E, 48], fp32, name="wv_sb")
            nc.sync.dma_start(out=wv_sb[:, :], in_=wv[:, :])
            xkT_sb = cpool.tile([E, NK], fp32, name="xkT_sb")
            nc.sync.dma_start(out=xkT_sb[:, :], in_=xkT[:, :])
            ident_sb = cpool.tile([E, E], fp32, name="ident_sb")
            nc.sync.dma_start(out=ident_sb[:, :], in_=ident[:, :])

            # ---------------- persistent work tensors ----------------
            qt = [wpool.tile([128, S], f32r, name=f"qt{h}") for h in range(2)]
            kt = [wpool.tile([128, NK], f32r, name=f"kt{h}") for h in range(2)]
            qhb = [wpool.tile([HD, S], bf16, name=f"qhb{h}") for h in range(2)]
            khb = [wpool.tile([HD, NK], bf16, name=f"khb{h}") for h in range(2)]
            vv = [wpool.tile([128, NKB, HD + 1], f32r, name=f"vv{h}") for h in range(2)]
            negp = [wpool.tile([128, NQB], fp32, name=f"negp{h}") for h in range(2)]
            nT8 = [
                [wpool.tile([NQB // 2, 128], f32r, name=f"nT8_{h}{hf}") for hf in range(2)]
                for h in range(2)
            ]
            ctxl = wpool.tile([49, S], fp32, name="ctxl")
            lsq = wpool.tile([128, 2 * HD], fp32, name="lsq")
            lisq = wpool.tile([128, 2 * HD], fp32, name="lisq")
            lbc = wpool.tile([48, S], fp32, name="lbc")
            out_sb = wpool.tile([64, S], fp32, name="out_sb")

            # special rows + zero fill for the unused contraction rows (both
            # sides: 0 * 0 avoids NaN from stale SBUF).  Rows 32:48/64:80/
            # 96:112 inside [18:128) are re-written by the split producers
            # below; WAW deps keep the order right.
            for h in range(2):
                nc.sync.dma_start(out=qt[h][16:17, :], in_=onesrow[:, :])
                nc.sync.dma_start(out=kt[h][16:17, :], in_=maskrow[:, :])
                nc.sync.dma_start(out=kt[h][17:18, :], in_=onesrow[:, 0:NK])
                nc.sync.dma_start(
                    out=qt[h][18:128, :], in_=zrow[0:1, 0:S].to_broadcast([110, S])
                )
                nc.sync.dma_start(
                    out=kt[h][18:128, :], in_=zrow[0:1, 0:NK].to_broadcast([110, NK])
                )
                nc.sync.dma_start(
                    out=vv[h][:, :, HD : HD + 1],
                    in_=onesrow[0:1, 0:NKB].to_broadcast([128, NKB]),
                )

            # ---------------- projections + splits ----------------
            # Q: 1/sqrt(hd) folded into wq host-side.  Per 1024-col half:
            def q_proj(half):
                cs = slice(1024 * half, 1024 * (half + 1))
                qt_ps = stpool.tile([48, 1024], fp32, name="qt_ps", tag="st")
                for c in range(2):
                    nc.tensor.matmul(
                        qt_ps[:, 512 * c : 512 * (c + 1)],
                        lhsT=wq_sb[:, :],
                        rhs=xT_sb[:, 1024 * half + 512 * c : 1024 * half + 512 * (c + 1)],
                        start=True,
                        stop=True,
                    )
                for h in range(2):
                    ps = qt_ps[32 * h : 32 * h + 16, :]
                    nc.scalar.copy(qhb[h][:, cs], ps)                 # bf16 round
                    nc.vector.tensor_copy(out=qt[h][0:16, cs], in_=qhb[h][:, cs])
                    nc.vector.tensor_tensor(
                        out=qt[h][64:80, cs], in0=ps, in1=qhb[h][:, cs], op=ALU.subtract
                    )
                    nc.sync.dma_start(out=qt[h][32:48, cs], in_=qt[h][0:16, cs])
                    nc.sync.dma_start(out=qt[h][96:112, cs], in_=qt[h][64:80, cs])

            def k_proj(o, n):
                cs = slice(o, o + n)
                kt_ps = stpool.tile([48, 512], fp32, name="kt_ps", tag="st")
                nc.tensor.matmul(
                    kt_ps[:, 0:n], lhsT=wk_sb[:, :], rhs=xkT_sb[:, cs], start=True, stop=True
                )
                for h in range(2):
                    ps = kt_ps[32 * h : 32 * h + 16, 0:n]
                    nc.scalar.copy(khb[h][:, cs], ps)                 # bf16 round
                    nc.vector.tensor_copy(out=kt[h][0:16, cs], in_=khb[h][:, cs])
                    nc.vector.tensor_tensor(
                        out=kt[h][32:48, cs], in0=ps, in1=khb[h][:, cs], op=ALU.subtract
                    )
                    nc.sync.dma_start(out=kt[h][64:80, cs], in_=kt[h][0:16, cs])
                    nc.sync.dma_start(out=kt[h][96:112, cs], in_=kt[h][32:48, cs])

            def v_iter(kb):
                v_ps = apool.tile([128, 48], fp32, name="v_ps", tag="a")
                nc.tensor.matmul(
                    v_ps[:, :],
                    lhsT=xkT_sb[:, 128 * kb : 128 * (kb + 1)],
                    rhs=wv_sb[:, :],
                    start=True,
                    stop=True,
                )
                nc.vector.tensor_copy(out=vv[0][:, kb, 0:HD], in_=v_ps[:, 0:16])
                nc.vector.tensor_copy(out=vv[1][:, kb, 0:HD], in_=v_ps[:, 32:48])

            # ---------------- pass A: subsampled row-max ----------------
            def a_iter(h, qb):
                sc = apool.tile([128, NA], fp32, name="sc", tag="a")
                nc.tensor.matmul(
                    sc[:, :],
                    lhsT=qt[h][0:17, 128 * qb : 128 * (qb + 1)],
                    rhs=kt[h][0:17, 0:NA],
                    start=True,
                    stop=True,
                )
                nc.vector.tensor_reduce(
                    negp[h][:, qb : qb + 1], sc[:, :], axis=AX.X, op=ALU.max, negate=True
                )

            def negm_half(h, hf):
                nq = NQB // 2
                ntp = apool.tile([nq, 128], fp32, name="ntp", tag="a")
                nc.tensor.transpose(
                    ntp[:, :], negp[h][:, nq * hf : nq * (hf + 1)], ident_sb[:, :]
                )
                nc.vector.tensor_copy(out=nT8[h][hf][:, :], in_=ntp[:, :])
                nc.sync.dma_start(
                    out=qt[h][17:18, 1024 * hf : 1024 * (hf + 1)].rearrange(
                        "a (b f) -> a b f", b=nq
                    ),
                    in_=nT8[h][hf][:, :],
                )

            # ---------------- pass B + ctx ----------------
            def b_iter(h, qh, kb, ctxc):
                st = stpool.tile([128, 1024], fp32, name="st", tag="st")
                lhs = kt[h][:, 128 * kb : 128 * (kb + 1)]
                for c in range(2):
                    nc.tensor.matmul(
                        st[:, 512 * c : 512 * (c + 1)],
                        lhsT=lhs,
                        rhs=qt[h][:, 1024 * qh + 512 * c : 1024 * qh + 512 * (c + 1)],
                        start=True,
                        stop=True,
                    )
                pt = ptpool.tile([128, 1024], f32r, name="pt", tag="pt")
                nc.scalar.activation(pt[:, :], st[:, :], AF.Exp)
                for c in range(2):
                    nc.tensor.matmul(
                        ctxc[c][0:17, :],
                        lhsT=vv[h][:, kb, :],
                        rhs=pt[:, 512 * c : 512 * (c + 1)],
                        start=(kb == 0),
                        stop=(kb == NKB - 1),
                    )

            def evac(h, qh, ctxc, last=False):
                eng = nc.vector if last else nc.scalar
                for c in range(2):
                    dst = ctxl[
                        32 * h : 32 * h + 17,
                        1024 * qh + 512 * c : 1024 * qh + 512 * (c + 1),
                    ]
                    if last:
                        nc.vector.tensor_copy(out=dst, in_=ctxc[c][0:17, :])
                    else:
                        nc.scalar.copy(dst, ctxc[c][0:17, :])

            def b_half(h, qh):
                return [
                    ctxpool.tile([17, 512], fp32, name=f"ctx{c}", tag="ctx")
                    for c in range(2)
                ]

            def finals(h, qh):
                # l half-row -> [128,8] square, parallel reciprocal, back out
                q0 = 1024 * qh
                co = 16 * h + 8 * qh
                nc.sync.dma_start(
                    out=lsq[:, co : co + 8],
                    in_=ctxl[32 * h + 16 : 32 * h + 17, q0 : q0 + 1024].rearrange(
                        "a (b f) -> a b f", b=128
                    ),
                )
                nc.vector.reciprocal(lisq[:, co : co + 8], lsq[:, co : co + 8])
                nc.sync.dma_start(
                    out=ldram[h : h + 1, q0 : q0 + 1024].rearrange(
                        "a (b f) -> a b f", b=128
                    ),
                    in_=lisq[:, co : co + 8],
                )
                nc.sync.dma_start(
                    out=lbc[32 * h : 32 * h + 16, q0 : q0 + 1024],
                    in_=ldram[h : h + 1, q0 : q0 + 1024].to_broadcast([HD, 1024]),
                )
                nc.vector.tensor_tensor(
                    out=out_sb[32 * h : 32 * h + 16, q0 : q0 + 1024],
                    in0=ctxl[32 * h : 32 * h + 16, q0 : q0 + 1024],
                    in1=lbc[32 * h : 32 * h + 16, q0 : q0 + 1024],
                    op=ALU.mult,
                )
                nc.sync.dma_start(
                    out=out_d[16 * h : 16 * h + 16, q0 : q0 + 1024],
                    in_=out_sb[32 * h : 32 * h + 16, q0 : q0 + 1024],
                )

            # ---------------- schedule ----------------
            q_proj(0)
            k_proj(0, 512)
            # A(h0) first half interleaved with remaining projections
            rest = [("q", 1, 0)] + [
                ("k", o, min(512, NK - o)) for o in range(512, NK, 512)
            ]
            vq = list(range(NKB))
            for qb in range(NQB // 2):
                a_iter(0, qb)
                if rest:
                    kind, a1, a2 = rest.pop(0)
                    q_proj(a1) if kind == "q" else k_proj(a1, a2)
                elif vq:
                    v_iter(vq.pop(0))
                if qb % 2 == 1 and vq:
                    v_iter(vq.pop(0))
            negm_half(0, 0)

            # B(h0,qh0) | A(h0) second half + V tail
            ctxc = b_half(0, 0)
            for kb in range(NKB):
                b_iter(0, 0, kb, ctxc)
                if kb < NQB // 2:
                    a_iter(0, NQB // 2 + kb)
                if vq:
                    v_iter(vq.pop(0))
            negm_half(0, 1)
            evac(0, 0, ctxc)

            # B(h0,qh1) | A(h1) first half
            ctxc = b_half(0, 1)
            for kb in range(NKB):
                b_iter(0, 1, kb, ctxc)
                if kb < NQB // 2:
                    a_iter(1, kb)
            negm_half(1, 0)
            evac(0, 1, ctxc)
            finals(0, 0)

            # B(h1,qh0) | A(h1) second half
            ctxc = b_half(1, 0)
            for kb in range(NKB):
                b_iter(1, 0, kb, ctxc)
                if kb < NQB // 2:
                    a_iter(1, NQB // 2 + kb)
            negm_half(1, 1)
            evac(1, 0, ctxc)
            finals(0, 1)

            # B(h1,qh1)
            ctxc = b_half(1, 1)
            for kb in range(NKB):
                b_iter(1, 1, kb, ctxc)
            finals(1, 0)
            evac(1, 1, ctxc, last=True)
            finals(1, 1)

    nc.finalize()
    return nc


def _prep_core_inputs(x, msk_add_full, w_query, w_key, w_value):
    """Build the 8 per-core input maps from full inputs.  Returns (maps, NKB)."""
    B = x.shape[0]
    onesrow = np.ones((1, S), dtype=np.float32)
    zrow = np.zeros((1, S), dtype=np.float32)
    identm = np.eye(E, dtype=np.float32)

    keeps = [np.flatnonzero(msk_add_full[b] == 0.0) for b in range(B)]
    max_nk = max(len(k) for k in keeps)
    assert max_nk >= NA, "row-max subsample needs >= NA valid keys"
    NKB = -(-max_nk // 128)  # ceil to 128
    NK = 128 * NKB

    per_batch = []
    for b in range(B):
        keep = keeps[b]
        nk = len(keep)
        xk = np.zeros((NK, E), dtype=np.float32)
        xk[:nk] = x[b][keep]
        maskrow = np.full((1, NK), NEG, dtype=np.float32)
        maskrow[0, :nk] = 0.0
        xTb = np.ascontiguousarray(x[b].T)
        xkTb = np.ascontiguousarray(xk.T)
        per_batch.append((xTb, xkTb, maskrow))

    in_maps = []
    for c in range(8):
        b = c // 4
        h0 = 2 * (c % 4)
        xTb, xkTb, maskrow = per_batch[b]

        def _pad48(w, scale=1.0):
            wc = np.zeros((E, 48), dtype=np.float32)
            wc[:, 0:16] = w[:, h0::8] * scale
            wc[:, 32:48] = w[:, h0 + 1 :: 8] * scale
            return wc

        in_maps.append(
            {
                "xT": xTb,
                "xkT": xkTb,
                "wq": _pad48(w_query, 0.25),  # 1/sqrt(hd) folded in (exact)
                "wk": _pad48(w_key),
                "wv": _pad48(w_value),
                "maskrow": maskrow,
                "onesrow": onesrow,
                "zrow": zrow,
                "ident": identm,
            }
        )
    return in_maps, NKB


def kernel(
    input_embeddings,
    token_attention_masks_source,
    token_attention_masks_target,
    masked,
    w_query,
    w_key,
    w_value,
):
    x = np.asarray(input_embeddings, dtype=np.float32)
    msk = np.asarray(token_attention_masks_source)
    wq_f = np.asarray(w_query, dtype=np.float32)
    wk_f = np.asarray(w_key, dtype=np.float32)
    wv_f = np.asarray(w_value, dtype=np.float32)
    assert int(np.asarray(masked)) == 0, "only the encoder (masked=0) path is supported"
    B = x.shape[0]
    assert x.shape == (2, S, E)

    msk_add = np.where(msk == 0, np.float32(NEG), np.float32(0.0))
    in_maps, NKB = _prep_core_inputs(x, msk_add, wq_f, wk_f, wv_f)

    if NKB not in _PROGS:
        _PROGS[NKB] = _build_program(NKB)
    nc = _PROGS[NKB]
    global _PROG
    _PROG = nc

    from concourse.bass_utils import run_bass_kernel_spmd

    res = run_bass_kernel_spmd(nc, in_maps, list(range(8)))

    out = np.empty((B, S, E), dtype=np.float32)
    for c in range(8):
        b = c // 4
        h0 = 2 * (c % 4)
        o = res.results[c]["out"]  # [32, 2048]
        out[b][:, h0::8] = o[0:16, :].T
        out[b][:, h0 + 1 :: 8] = o[16:32, :].T

    # The device row-max is a lower bound from a 512-key subsample; rows where
    # the true max exceeds it by >~88 overflow exp to inf (-> inf or NaN or,
    # when only the denominator overflows, an exact-zero vector).  Those rows
    # are deterministic and rare (<1%); recompute them exactly on host.
    for b in range(B):
        for h in range(8):
            hv = out[b][:, h::8]  # [S, 16]
            bad = ~np.isfinite(hv).all(axis=1) | (hv == 0.0).all(axis=1)
            if not bad.any():
                continue
            rows = np.flatnonzero(bad)
            xb = x[b].astype(np.float64)
            qh = (xb[rows] @ wq_f[:, h::8].astype(np.float64)) * 0.25
            kh = xb @ wk_f[:, h::8].astype(np.float64)
            vh = xb @ wv_f[:, h::8].astype(np.float64)
            sc = qh @ kh.T + msk_add[b][None, :].astype(np.float64)
            sc -= sc.max(axis=1, keepdims=True)
            p = np.exp(sc)
            p /= p.sum(axis=1, keepdims=True)
            out[b][rows, h::8] = (p @ vh).astype(np.float32)
    return out


_PROG = None
